# revision 27
# baseline (speedup 1.0000x reference)
"""LocalConvolution via a pad-free pixel-pair segmented-MAC DVE op (fp16, 2x).

Sharding: 8 cores = (batch n in 4) x (H-half in 2); per core [64, 64, 128].
Partitions = (weight-channel j in 8) x (4-row block pc in 16).

Per (kernel-row i, pixel-parity): one MAC_PAIR10_ANT scan covering TWO
channel groups (264 pixel pairs per partition). The x operand is a
host-repacked two-group window; the weight tile is identical across
groups and is re-streamed in place via a stride-0 outer AP dim
([[0, 2], [1, 1320]]). Each pair (A, B) consumes 10 weights (5 per
pixel, zero padding eliminated) and one shared 10-element x window; the
two 5-tap sums share x reads via swap-flop latches inside a 5-state
(2x) / 11-state (1x) uOp FSM, emitting the A/B sums as one fp16 write
pair. The 5 row-sums per parity are summed on GPSIMD; the final block
runs as two single-group sub-blocks with an incremental DVE combine so
only one narrow add + store trail the last scan. fp16 outputs are
reassembled and upcast on the host.

Weight order per pair: [A0 A1 A2 A3 A4  B2 B0 B1 B3 B4] where Aj/Bj is
the j-th column tap of the even/odd pair member. x window per pair k is
x[4k .. 4k+9] (elements 7..9 are consumed for stream lockstep but unused;
B only needs x[4k+2 .. 4k+6]).
"""

import os

import numpy as np

try:
    import concourse.bass as bass
except ImportError:
    import sys

    for p in ("/opt/trn_rl_repo", "/root/.axon_site/_ro/trn_rl_repo"):
        if p not in sys.path:
            sys.path.insert(0, p)
    import concourse.bass as bass
import concourse.mybir as mybir
from concourse import tile
from concourse.bass_utils import run_bass_kernel_spmd


def _split_multi_waits(nc):
    n_split = 0
    for fn in nc.m.functions:
        for bb in fn.blocks:
            new_insts = []
            for inst in bb.instructions:
                si = inst.sync_info
                if si is not None and len(si.on_wait) > 1:
                    waits = list(si.on_wait)
                    for k, w in enumerate(waits[:-1]):
                        n_split += 1
                        new_insts.append(
                            mybir.InstNoOp(
                                name=f"{inst.name}_w{k}",
                                engine=inst.engine,
                                sync_info=mybir.SyncInfo(
                                    on_wait=[w], on_update=[]
                                ),
                                bass_nofuse=True,
                            )
                        )
                    inst.sync_info = mybir.SyncInfo(
                        on_wait=[waits[-1]], on_update=list(si.on_update)
                    )
                new_insts.append(inst)
            bb.instructions = new_insts
    return n_split


def _register_mac_pair10():
    from concourse import dve_ops
    from concourse.dve_spec import AluOp as SAluOp, Spec, Src0, Src1, scan
    from concourse.dve_table_gen import dve_ver_for
    from concourse.dve_uop import (
        ENABLE,
        AluInp,
        AluOp,
        DelayInp,
        DveOpSpec,
        InpSel,
        OutPath,
        OutSel,
        Trigger,
        UopConfig,
    )

    name = "MAC_PAIR10_ANT"
    PD = AluInp.PREV_DELAY_0

    # --- 2X_1PORT program --------------------------------------------------
    # Chains: 0 = x_lo, 1 = w_lo, 2 = x_hi, 3 = w_hi, 4 = product capture,
    # 5 = late capture (B2 / s / A-emit). Accumulators: A in st6's out flop,
    # B in st7's (CURR_ALU_OUT temporal feedback). Swap latches: x2@st3,
    # x3@st4 (u2), x5@st5 (u3), x6@st2 (u4). Per 5-issue period (one pair):
    #   u1 (x0,x1 | wA0,wA1): s01 = x0w0+x1w1; A <- s01 (seed)
    #   u2 (x2,x3 | wA2,wA3): A += s23; latch x2, x3
    #   u3 (x4,x5 | wA4,wB2): A += x4*wA4 (final); B <- x4*wB2 (seed); latch x5
    #   u4 (x6,x7 | wB0,wB1): B += x2*wB0 + x3*wB1; stash A into lane5@st7
    #   u5 (x8,x9 | wB3,wB4): B += x5*wB3 + x6*wB4; emit (A, B) as LO/HI
    def _u2x(kind):
        u = UopConfig()
        u.enable_input(InpSel.SRC_0, 1)  # -> chain 0 (x_lo)
        u.enable_input(InpSel.SRC_1, 2)  # -> chain 1 (w_lo)
        u.enable_input(InpSel.SRC_0_HI, 3)  # -> chain 2 (x_hi)
        u.enable_input(InpSel.SRC_1_HI, 4)  # -> chain 3 (w_hi)
        u.require_inp0 = ENABLE
        u.require_inp1 = ENABLE
        dp = u.datapath_config
        if kind in ("u1", "u2"):
            dp[0].enable_alu(AluOp.MULTIPLY, AluInp(PD + 0), AluInp(PD + 1))
            dp[0].pass_through_delay(2, 3)  # x_hi, w_hi onward to st1's mul
            dp[1].enable_alu(AluOp.MULTIPLY, AluInp(PD + 2), AluInp(PD + 3))
            dp[1].enable_delay_from_src(DelayInp.PREV_ALU_OUT, 4)
            dp[2].enable_alu(AluOp.ADD, AluInp.PREV_ALU_OUT, AluInp(PD + 4))
            if kind == "u2":
                # carry x_lo to st3, x_hi to st4 for the latches
                dp[0].pass_through_delay(0)
                dp[1].pass_through_delay(0, 2)
                dp[2].pass_through_delay(0, 2)
                dp[3].enable_alu(
                    AluOp.BYPASS, AluInp.PREV_ALU_OUT, AluInp(PD + 0)
                )
                dp[3].swap_enable = ENABLE  # swap@st3 <- x2
                dp[3].pass_through_delay(2)
                dp[4].enable_alu(
                    AluOp.BYPASS, AluInp.PREV_ALU_OUT, AluInp(PD + 2)
                )
                dp[4].swap_enable = ENABLE  # swap@st4 <- x3
            else:
                dp[3].pass_through_alu()
                dp[4].pass_through_alu()
            dp[5].pass_through_alu()
            if kind == "u1":
                dp[6].enable_alu(
                    AluOp.BYPASS, AluInp.PREV_ALU_OUT, AluInp.PREV_ALU_OUT
                )  # A <- s01
            else:
                dp[6].enable_alu(
                    AluOp.ADD, AluInp.CURR_ALU_OUT, AluInp.PREV_ALU_OUT
                )  # A += s23
        elif kind == "u3":
            # st0: A4 = x4*wA4; st1: B2 = x4*wB2 (x_lo reused on both muls)
            dp[0].enable_alu(AluOp.MULTIPLY, AluInp(PD + 0), AluInp(PD + 1))
            dp[0].pass_through_delay(0, 2, 3)
            dp[1].enable_alu(AluOp.MULTIPLY, AluInp(PD + 0), AluInp(PD + 3))
            dp[1].enable_delay_from_src(DelayInp.PREV_ALU_OUT, 4)  # A4
            dp[1].pass_through_delay(2)
            for st in (2, 3, 4):
                dp[st].pass_through_alu()  # pass B2 down
                dp[st].pass_through_delay(2, 4)
            dp[5].enable_alu(
                AluOp.BYPASS, AluInp.PREV_ALU_OUT, AluInp(PD + 2)
            )
            dp[5].swap_enable = ENABLE  # swap@st5 <- x5
            dp[5].pass_through_delay(4)
            dp[6].enable_alu(AluOp.ADD, AluInp.CURR_ALU_OUT, AluInp(PD + 4))
            dp[6].enable_delay_from_src(DelayInp.PREV_ALU_OUT, 5)  # B2
            dp[7].enable_alu(AluOp.BYPASS, AluInp(PD + 5), AluInp(PD + 5))
            # st7 out flop <- B2 (B seed)
        elif kind == "u4":
            # w pair (wB0, wB1); x pair (x6, x7): latch x6, x7 unused.
            dp[0].pass_through_delay(0, 1, 3)
            dp[1].pass_through_delay(0, 1, 3)
            dp[2].enable_alu(
                AluOp.BYPASS, AluInp.PREV_ALU_OUT, AluInp(PD + 0)
            )
            dp[2].swap_enable = ENABLE  # swap@st2 <- x6
            dp[2].pass_through_delay(1, 3)
            dp[3].enable_alu(AluOp.MULTIPLY, AluInp.CURR_SWAP_OUT, AluInp(PD + 1))
            dp[3].pass_through_delay(3)  # B0 = x2*wB0
            dp[4].enable_alu(AluOp.MULTIPLY, AluInp.CURR_SWAP_OUT, AluInp(PD + 3))
            dp[4].enable_delay_from_src(DelayInp.PREV_ALU_OUT, 4)  # B0
            # B1 = x3*wB1
            dp[5].enable_alu(AluOp.ADD, AluInp.PREV_ALU_OUT, AluInp(PD + 4))
            dp[6].enable_delay_from_src(DelayInp.PREV_ALU_OUT, 5)  # s; A held
            dp[7].enable_alu(AluOp.ADD, AluInp.CURR_ALU_OUT, AluInp(PD + 5))
            dp[7].enable_delay_from_src(DelayInp.PREV_ALU_OUT, 5)  # A -> lane5@st7
        elif kind == "u5":
            # w pair (wB3, wB4); x pair consumed but unused.
            dp[0].pass_through_delay(1, 3)
            dp[1].pass_through_delay(1, 3)
            dp[2].enable_alu(AluOp.MULTIPLY, AluInp.CURR_SWAP_OUT, AluInp(PD + 3))
            dp[2].pass_through_delay(1)  # B4 = x6*wB4
            dp[3].pass_through_alu()
            dp[3].pass_through_delay(1)
            dp[4].pass_through_alu()
            dp[4].pass_through_delay(1)
            dp[5].enable_alu(AluOp.MULTIPLY, AluInp.CURR_SWAP_OUT, AluInp(PD + 1))
            dp[5].enable_delay_from_src(DelayInp.PREV_ALU_OUT, 4)  # B4
            # B3 = x5*wB3
            dp[6].enable_alu(AluOp.ADD, AluInp.PREV_ALU_OUT, AluInp(PD + 4))
            # s2 = B3+B4 (clobbers A flop; A already stashed in lane5@st7)
            dp[7].enable_alu(AluOp.ADD, AluInp.CURR_ALU_OUT, AluInp.PREV_ALU_OUT)
            u.enable_output(OutSel.DELAY_5, OutPath.WR0_LO)  # A
            u.enable_output(OutSel.ALU_OUT, OutPath.WR0_HI)  # B
        return u

    def _chain2x(u, succ):
        u.trigger = (Trigger.SRC_TENSOR_DONE, Trigger.COUNT, Trigger.NONE)
        u.next_uop = (0, succ, 0)
        u.repeat_count = 1
        return u

    # index 0 is the entry copy of u1 (index 0 is also IDLE as a next_uop
    # target, so the loop body lives at 1..5)
    two_uops = [
        _chain2x(_u2x("u1"), 2),
        _chain2x(_u2x("u1"), 2),
        _chain2x(_u2x("u2"), 3),
        _chain2x(_u2x("u3"), 4),
        _chain2x(_u2x("u4"), 5),
        _chain2x(_u2x("u5"), 1),
    ]

    # --- 1X program (fallback; also what runs if alignment breaks) ---------
    # Chains: 0 = x, 1 = w, 4/5 = captures. Swap latches: x2@st1, x3@st2,
    # x4@st3, x5@st4, x6@st5. A accumulates in st6, B in st7. A is emitted
    # at i4 (via st7 bypass), B at i9.
    def _u1x(kind):
        u = UopConfig()
        u.enable_input(InpSel.SRC_0, 1)  # -> chain 0 (x)
        u.enable_input(InpSel.SRC_1, 2)  # -> chain 1 (w)
        u.require_inp0 = ENABLE
        u.require_inp1 = ENABLE
        dp = u.datapath_config
        if kind in ("i0", "i1", "i2", "i3", "i4"):
            dp[0].enable_alu(AluOp.MULTIPLY, AluInp(PD + 0), AluInp(PD + 1))
            latch_st = {"i2": 1, "i3": 2, "i4": 3}.get(kind)
            if latch_st is not None:
                for st in range(latch_st):
                    dp[st].pass_through_delay(0)
            for st in range(1, 6):
                if st == latch_st:
                    dp[st].enable_alu(
                        AluOp.BYPASS, AluInp.PREV_ALU_OUT, AluInp(PD + 0)
                    )
                    dp[st].swap_enable = ENABLE
                else:
                    dp[st].pass_through_alu()
            if kind == "i0":
                dp[6].enable_alu(
                    AluOp.BYPASS, AluInp.PREV_ALU_OUT, AluInp.PREV_ALU_OUT
                )
            else:
                dp[6].enable_alu(
                    AluOp.ADD, AluInp.CURR_ALU_OUT, AluInp.PREV_ALU_OUT
                )
            if kind == "i4":
                # A final: mirror it into st7's flop and emit
                dp[7].enable_alu(
                    AluOp.BYPASS, AluInp.PREV_ALU_OUT, AluInp.PREV_ALU_OUT
                )
                u.enable_output(OutSel.ALU_OUT, OutPath.WR0_LO)
        elif kind == "i5":
            # B2 = x4*wB2 at st3; latch x5@st4; B <- B2 (seed)
            for st in (0, 1, 2):
                dp[st].pass_through_delay(0, 1)
            dp[3].enable_alu(AluOp.MULTIPLY, AluInp.CURR_SWAP_OUT, AluInp(PD + 1))
            dp[3].pass_through_delay(0)
            dp[4].enable_alu(
                AluOp.BYPASS, AluInp.PREV_ALU_OUT, AluInp(PD + 0)
            )
            dp[4].swap_enable = ENABLE
            dp[5].pass_through_alu()
            dp[6].enable_delay_from_src(DelayInp.PREV_ALU_OUT, 5)  # B2; A held
            dp[7].enable_alu(AluOp.BYPASS, AluInp(PD + 5), AluInp(PD + 5))
        elif kind == "i6":
            # B0 = x2*wB0 at st1; latch x6@st5; B += B0
            dp[0].pass_through_delay(0, 1)
            dp[1].enable_alu(AluOp.MULTIPLY, AluInp.CURR_SWAP_OUT, AluInp(PD + 1))
            dp[1].pass_through_delay(0)
            for st in (2, 3, 4):
                dp[st].pass_through_alu()
                dp[st].pass_through_delay(0)
            dp[5].enable_alu(
                AluOp.BYPASS, AluInp.PREV_ALU_OUT, AluInp(PD + 0)
            )
            dp[5].swap_enable = ENABLE
            dp[6].enable_delay_from_src(DelayInp.PREV_ALU_OUT, 5)
            dp[7].enable_alu(AluOp.ADD, AluInp.CURR_ALU_OUT, AluInp(PD + 5))
        elif kind in ("i7", "i8", "i9"):
            mul_st = {"i7": 2, "i8": 4, "i9": 5}[kind]
            for st in range(mul_st):
                dp[st].pass_through_delay(1)
            dp[mul_st].enable_alu(
                AluOp.MULTIPLY, AluInp.CURR_SWAP_OUT, AluInp(PD + 1)
            )
            for st in range(mul_st + 1, 6):
                dp[st].pass_through_alu()
            dp[6].enable_delay_from_src(DelayInp.PREV_ALU_OUT, 5)
            dp[7].enable_alu(AluOp.ADD, AluInp.CURR_ALU_OUT, AluInp(PD + 5))
            if kind == "i9":
                u.enable_output(OutSel.ALU_OUT, OutPath.WR0_LO)
        return u

    def _chain1x(u, succ):
        u.trigger = (Trigger.SRC_TENSOR_DONE, Trigger.COUNT, Trigger.NONE)
        u.next_uop = (0, succ, 0)
        u.repeat_count = 1
        return u

    kinds1x = ["i0", "i0", "i1", "i2", "i3", "i4", "i5", "i6", "i7", "i8", "i9"]
    base_uops = [
        _chain1x(_u1x(k), 2 if idx == 0 else (idx + 1) if idx < 10 else 1)
        for idx, k in enumerate(kinds1x)
    ]
    # table-gen requires equal variant lengths; pad 2x with unreachable slots
    two_uops = two_uops + [UopConfig() for _ in range(len(base_uops) - len(two_uops))]

    def _ref(in0, in1, s0, s1, imm2):
        p = in0.shape[0]
        x = np.asarray(in0, np.float32).reshape(p, -1, 10)
        w = np.asarray(in1, np.float32).reshape(p, -1, 10)
        a = (x[..., 0:5] * w[..., 0:5]).sum(axis=-1)
        b = (
            x[..., 4] * w[..., 5]
            + x[..., 2] * w[..., 6]
            + x[..., 3] * w[..., 7]
            + x[..., 5] * w[..., 8]
            + x[..., 6] * w[..., 9]
        )
        out = np.stack([a, b], axis=-1).reshape(p, -1)
        return out

    spec = Spec(body=scan(SAluOp.ADD, Src0 * Src1), reference=_ref)
    if name in dve_ops._SUB_OPCODE_FOR_NAME:
        row = dve_ops._SUB_OPCODE_FOR_NAME[name]
        op = next(o for o in dve_ops.OPS if o.name == name)
    else:
        row = dve_ops._CUSTOM_DVE_ROW_BASE + len(dve_ops.OPS)
        assert row < 0x20
        op = None
    shas = {}
    for ver in {dve_ver_for("TRN2"), "v3", "v4"}:
        compiled = DveOpSpec(
            name=name,
            opcode=row,
            uops=base_uops,
            uops_2x=two_uops,
            rd1_en=True,
            perf_max=1,
        )
        dve_ops._COMPILE_CACHE[(name, ver)] = compiled
        shas[ver] = compiled.sha(ver)
    if op is None:
        op = dve_ops.DveOp(name, spec, subdim=True, uops_sha=shas)
        dve_ops.OPS.append(op)
        dve_ops.CUSTOM_DVE_SPECS[name] = spec
        dve_ops._SUB_OPCODE_FOR_NAME[name] = row
    else:
        op.uops_sha.clear()
        op.uops_sha.update(shas)
    return op


def _set_perf_max(nc, val):
    for fn in nc.m.functions:
        for bb in fn.blocks:
            for inst in bb.instructions:
                if isinstance(inst, mybir.InstCustomDveAnt):
                    inst.perf_max = val


N, C, H, W = 4, 64, 128, 128
K, PAD, CW = 5, 2, 8
HO, WO = 128, 128
RH = 64
WP = W + 2 * PAD  # 132
NJ, NPC, RB = 8, 16, 4
QP = RB * WP // 2  # 264 pixels per parity per partition per group
NPAIR = QP // 2  # 132 pixel pairs
WSEG = 10  # weights per pair (pad-free)
WFREE = NPAIR * WSEG  # 1320
XL = (RB + K) * WP  # 1188
GG = 2  # channel groups merged per scan (weights re-streamed via stride-0)
NGG = 8 // GG  # merged blocks per parity
XW = 4 * NPAIR  # per-group x window inside a merged tile (528)
XQ = XW + XW + 8  # merged two-group x window (1064)
F16 = mybir.dt.float16

PHASE = int(os.environ.get("LC_PHASE", "2"))


def _build_program(phase=PHASE, repeat=1):
    nc = bass.Bass()
    mac = _register_mac_pair10()
    # xq: per (par, i, gg) a two-group merged x window; group 2*gg's 528
    # elements then group 2*gg+1's window (534 + pad)
    xq_d = nc.declare_dram_parameter(
        "xq", [2, K, NGG, 128, XQ], F16, isOutput=False
    )
    ws_d = nc.declare_dram_parameter("ws", [K, 2, 128, WFREE], F16, isOutput=False)
    out_d = nc.declare_dram_parameter("out", [8, 2, 128, QP], F16, isOutput=True)

    xq_a = xq_d[:]
    ws_a = ws_d[:]
    out_a = out_d[:]

    with tile.TileContext(nc) as tc:
        with (
            tc.tile_pool(name="wpool", bufs=1) as wpool,
            tc.tile_pool(name="xpool", bufs=1) as xpool,
            tc.tile_pool(name="opool", bufs=4) as opool,
            tc.tile_pool(name="ogpool", bufs=4) as ogpool,
            tc.tile_pool(name="tpool", bufs=3) as tpool,
        ):
            # All input DMAs are pre-emitted in first-use order (blocks run
            # parity-major); w(i) and xq(i, gg0/gg1) interleave so the first
            # two merged blocks' scans are paced by arrivals
            w_tiles = {}
            x_tiles = {}

            def _load_w(i, par):
                wt = wpool.tile([128, WFREE], F16, tag=f"w{i}_{par}")
                nc.sync.dma_start(
                    wt[:],
                    ws_a.__replace__(
                        ap=[[WFREE, 128], [1, WFREE]],
                        offset=(i * 2 + par) * 128 * WFREE,
                    ),
                )
                w_tiles[(i, par)] = wt

            def _load_xq(par, i, gg):
                xt = xpool.tile([128, XQ], F16, tag=f"xq{par}_{i}_{gg}")
                nc.sync.dma_start(
                    xt[:],
                    xq_a.__replace__(
                        ap=[[XQ, 128], [1, XQ]],
                        offset=((par * K + i) * NGG + gg) * 128 * XQ,
                    ),
                )
                x_tiles[(par, i, gg)] = xt

            _load_w(0, 0)
            _load_xq(0, 0, 0)
            _load_w(1, 0)
            _load_xq(0, 1, 0)
            _load_xq(0, 0, 1)
            for i in range(2, K):
                _load_w(i, 0)
                _load_xq(0, i, 0)
                _load_xq(0, i - 1, 1)
            _load_xq(0, K - 1, 1)
            for gg in range(2, NGG):
                for i in range(K):
                    _load_xq(0, i, gg)
            for i in range(K):
                _load_w(i, 1)
            for gg in range(NGG):
                for i in range(K):
                    _load_xq(1, i, gg)

            def _scan(par, gg, i, gsub=None):
                """Merged two-group scan, or a single group of the merged
                tile when gsub is given (used to keep the tail combine
                small on the final block)."""
                if gsub is None:
                    o = opool.tile([128, 2 * QP], F16, tag=f"o{par}_{i}")
                    npair, xoff, woff, ov = GG * NPAIR, 0, 0, None
                else:
                    o = opool.tile([128, QP], F16, tag=f"os{par}_{i}")
                    npair, xoff = NPAIR, gsub * XW
                xa = x_tiles[(par, i, gg)][:]
                in0 = xa.__replace__(
                    ap=[xa.ap[0], [4, npair], [1, WSEG]],
                    offset=xa.offset + xoff,
                )
                wa = w_tiles[(i, par)][:]
                wap = (
                    [wa.ap[0], [1, WFREE]]
                    if gsub is not None
                    else [wa.ap[0], [0, GG], [1, WFREE]]
                )
                in1 = wa.__replace__(ap=wap, offset=wa.offset)
                nc.vector._custom_dve(mac, out=o[:], in0=in0, in1=in1)
                return o

            # first two even-parity merged blocks: interleave scan emission
            # so block-1 scans fill block-0's weight-arrival stalls
            pre = {0: [], 1: []}
            order = [(0, 0), (0, 1), (1, 0), (0, 2), (1, 1), (0, 3), (1, 2),
                     (0, 4), (1, 3), (1, 4)]
            for gg, i in order:
                pre[gg].append((i, _scan(0, gg, i)))
            for gg in (0, 1):
                pre[gg] = [o for _, o in sorted(pre[gg])]

            def _store(par, g, og, col):
                nc.sync.dma_start(
                    out_a.__replace__(
                        ap=[[QP, 128], [1, QP]],
                        offset=(g * 2 + par) * 128 * QP,
                    ),
                    og[:, col : col + QP],
                )

            for par in range(2):
                for gg in range(repeat * NGG):
                    gg = gg % NGG
                    if par == 1 and gg == NGG - 1:
                        # final block: two single-group sub-blocks so only a
                        # narrow combine trails the last scan; the first
                        # sub-block's combine runs on Pool in parallel
                        for gsub in range(GG):
                            og = ogpool.tile([128, QP], F16, tag="ogf")
                            if gsub == GG - 1:
                                # combine incrementally on the DVE between
                                # scans; one add + store trail the last scan
                                o0 = _scan(par, gg, 0, gsub)
                                o1 = _scan(par, gg, 1, gsub)
                                nc.vector.tensor_add(
                                    og[:, :], o0[:, :], o1[:, :]
                                )
                                for i in range(2, K):
                                    oi = _scan(par, gg, i, gsub)
                                    nc.vector.tensor_add(
                                        og[:, :], og[:, :], oi[:, :]
                                    )
                            else:
                                os_ = [
                                    _scan(par, gg, i, gsub) for i in range(K)
                                ]
                                t2 = tpool.tile([128, QP], F16, tag="tf")
                                nc.gpsimd.tensor_add(
                                    og[:, :], os_[0][:, :], os_[1][:, :]
                                )
                                nc.gpsimd.tensor_add(
                                    t2[:, :], os_[2][:, :], os_[3][:, :]
                                )
                                nc.gpsimd.tensor_add(
                                    og[:, :], og[:, :], t2[:, :]
                                )
                                nc.gpsimd.tensor_add(
                                    og[:, :], og[:, :], os_[4][:, :]
                                )
                            _store(par, gg * GG + gsub, og, 0)
                        continue
                    og = ogpool.tile([128, 2 * QP], F16, tag=f"og{par}")
                    if par == 0 and gg in (0, 1):
                        os_ = pre[gg]
                    else:
                        os_ = [_scan(par, gg, i) for i in range(K)]
                    t2 = tpool.tile([128, 2 * QP], F16, tag=f"t{par}")
                    eng = nc.gpsimd
                    eng.tensor_add(og[:, :], os_[0][:, :], os_[1][:, :])
                    eng.tensor_add(t2[:, :], os_[2][:, :], os_[3][:, :])
                    eng.tensor_add(og[:, :], og[:, :], t2[:, :])
                    eng.tensor_add(og[:, :], og[:, :], os_[4][:, :])
                    for gsub in range(GG):
                        _store(par, gg * GG + gsub, og, gsub * QP)
    _set_perf_max(nc, 1)
    mybir.codegen_inst_isa_subclasses(nc)
    _split_multi_waits(nc)
    return nc


def _shard_inputs(input, weight):
    input = np.asarray(input, dtype=np.float32)
    weight = np.asarray(weight, dtype=np.float32)
    in_maps = []
    for n in range(N):
        xp = np.pad(input[n], ((0, 0), (PAD, PAD + 1), (PAD, PAD)))  # [64,133,132]
        sw = np.lib.stride_tricks.sliding_window_view(xp, (RB + K), axis=1)
        sw = np.transpose(sw, (0, 1, 3, 2))  # [c, row0, 9, 132]
        wv = weight[n].reshape(NJ, K, K, HO, WO)
        for half in range(2):
            r0 = RH * half
            idx = r0 + np.arange(NPC) * RB
            slab = sw[:, idx]  # [64, 16, 9, 132]
            xe = np.ascontiguousarray(slab.reshape(C // CW, NJ, NPC, XL))
            xe = xe.reshape(8, 128, XL)
            xo = np.zeros_like(xe)
            xo[..., :-1] = xe[..., 1:]
            xs = np.stack([xe, xo], axis=1).astype(np.float16)  # [8, 2, 128, XL]

            warr = wv[:, :, :, r0 : r0 + RH, :].reshape(
                NJ, K, K, NPC, RB, WO // 2, 2
            )  # [j, i, jj, pc, rr, m, par]
            # taps per (i, par, j, pc, rr, within-parity pixel m, col-tap jj)
            wt = np.zeros((K, 2, NJ, NPC, RB, WP // 2, K), np.float32)
            wt[:, :, :, :, :, : WO // 2, :] = np.transpose(
                warr, (1, 6, 0, 3, 4, 5, 2)
            )
            # pack per pixel-pair: [A0..A4, B2, B0, B1, B3, B4]
            wpair = wt.reshape(K, 2, NJ, NPC, RB, WP // 4, 2, K)
            wpp = np.empty((K, 2, NJ, NPC, RB, WP // 4, WSEG), np.float32)
            wpp[..., 0:5] = wpair[..., 0, :]
            wpp[..., 5] = wpair[..., 1, 2]
            wpp[..., 6] = wpair[..., 1, 0]
            wpp[..., 7] = wpair[..., 1, 1]
            wpp[..., 8] = wpair[..., 1, 3]
            wpp[..., 9] = wpair[..., 1, 4]
            ws = wpp.reshape(K, 2, 128, WFREE).astype(np.float16)

            # merged two-group x windows: xq[par, i, gg] =
            #   [group 2gg's window (528) | group 2gg+1's window (536)]
            xq = np.zeros((2, K, NGG, 128, XQ), np.float16)
            for par in range(2):
                for i in range(K):
                    for gg in range(NGG):
                        xq[par, i, gg, :, :XW] = xs[
                            2 * gg, par, :, WP * i : WP * i + XW
                        ]
                        xq[par, i, gg, :, XW : XW + XW + 8] = xs[
                            2 * gg + 1, par, :, WP * i : WP * i + XW + 8
                        ]
            in_maps.append({"xq": xq, "ws": ws})
    return in_maps


def kernel(input, weight):
    nc = _build_program(PHASE)
    in_maps = _shard_inputs(input, weight)
    res = run_bass_kernel_spmd(nc, in_maps, list(range(8)))
    out = np.empty((N, C, HO, WO), dtype=np.float32)
    for k in range(8):
        n, half = divmod(k, 2)
        o = np.asarray(res.results[k]["out"], dtype=np.float32)
        o = o.reshape(8, 2, NJ, NPC, RB, WP // 2)[..., : WO // 2]
        o = np.transpose(o, (0, 2, 3, 4, 5, 1))  # [g, j, pc, rr, m, par]
        out[n, :, RH * half : RH * (half + 1), :] = o.reshape(C, RH, WO)
    return out


# revision 32
# speedup vs baseline: 1.0136x; 1.0136x over previous
"""LocalConvolution via a pad-free pixel-pair segmented-MAC DVE op (fp16, 2x).

Sharding: 8 cores = (batch n in 4) x (H-half in 2); per core [64, 64, 128].
Partitions = (weight-channel j in 8) x (4-row block pc in 16).

Per (kernel-row i, pixel-parity): one MAC_PAIR10_ANT scan covering TWO
channel groups (264 pixel pairs per partition). The x operand is a
host-repacked two-group window; the weight tile is identical across
groups and is re-streamed in place via a stride-0 outer AP dim
([[0, 2], [1, 1320]]). Each pair (A, B) consumes 10 weights (5 per
pixel, zero padding eliminated) and one shared 10-element x window; the
two 5-tap sums share x reads via swap-flop latches inside a 5-state
(2x) / 11-state (1x) uOp FSM, emitting the A/B sums as one fp16 write
pair. The 5 row-sums per parity are summed on GPSIMD; the final block
runs as two single-group sub-blocks with an incremental DVE combine so
only one narrow add + store trail the last scan. fp16 outputs are
reassembled and upcast on the host.

Weight order per pair: [A0 A1 A2 A3 A4  B2 B0 B1 B3 B4] where Aj/Bj is
the j-th column tap of the even/odd pair member. x window per pair k is
x[4k .. 4k+9] (elements 7..9 are consumed for stream lockstep but unused;
B only needs x[4k+2 .. 4k+6]).
"""

import os

import numpy as np

try:
    import concourse.bass as bass
except ImportError:
    import sys

    for p in ("/opt/trn_rl_repo", "/root/.axon_site/_ro/trn_rl_repo"):
        if p not in sys.path:
            sys.path.insert(0, p)
    import concourse.bass as bass
import concourse.mybir as mybir
from concourse import tile
from concourse.bass_utils import run_bass_kernel_spmd


def _split_multi_waits(nc):
    n_split = 0
    for fn in nc.m.functions:
        for bb in fn.blocks:
            new_insts = []
            for inst in bb.instructions:
                si = inst.sync_info
                if si is not None and len(si.on_wait) > 1:
                    waits = list(si.on_wait)
                    for k, w in enumerate(waits[:-1]):
                        n_split += 1
                        new_insts.append(
                            mybir.InstNoOp(
                                name=f"{inst.name}_w{k}",
                                engine=inst.engine,
                                sync_info=mybir.SyncInfo(
                                    on_wait=[w], on_update=[]
                                ),
                                bass_nofuse=True,
                            )
                        )
                    inst.sync_info = mybir.SyncInfo(
                        on_wait=[waits[-1]], on_update=list(si.on_update)
                    )
                new_insts.append(inst)
            bb.instructions = new_insts
    return n_split


def _register_mac_pair10():
    from concourse import dve_ops
    from concourse.dve_spec import AluOp as SAluOp, Spec, Src0, Src1, scan
    from concourse.dve_table_gen import dve_ver_for
    from concourse.dve_uop import (
        ENABLE,
        AluInp,
        AluOp,
        DelayInp,
        DveOpSpec,
        InpSel,
        OutPath,
        OutSel,
        Trigger,
        UopConfig,
    )

    name = "MAC_PAIR10_ANT"
    PD = AluInp.PREV_DELAY_0

    # --- 2X_1PORT program --------------------------------------------------
    # Chains: 0 = x_lo, 1 = w_lo, 2 = x_hi, 3 = w_hi, 4 = product capture,
    # 5 = late capture (B2 / s / A-emit). Accumulators: A in st6's out flop,
    # B in st7's (CURR_ALU_OUT temporal feedback). Swap latches: x2@st3,
    # x3@st4 (u2), x5@st5 (u3), x6@st2 (u4). Per 5-issue period (one pair):
    #   u1 (x0,x1 | wA0,wA1): s01 = x0w0+x1w1; A <- s01 (seed)
    #   u2 (x2,x3 | wA2,wA3): A += s23; latch x2, x3
    #   u3 (x4,x5 | wA4,wB2): A += x4*wA4 (final); B <- x4*wB2 (seed); latch x5
    #   u4 (x6,x7 | wB0,wB1): B += x2*wB0 + x3*wB1; stash A into lane5@st7
    #   u5 (x8,x9 | wB3,wB4): B += x5*wB3 + x6*wB4; emit (A, B) as LO/HI
    def _u2x(kind):
        u = UopConfig()
        u.enable_input(InpSel.SRC_0, 1)  # -> chain 0 (x_lo)
        u.enable_input(InpSel.SRC_1, 2)  # -> chain 1 (w_lo)
        u.enable_input(InpSel.SRC_0_HI, 3)  # -> chain 2 (x_hi)
        u.enable_input(InpSel.SRC_1_HI, 4)  # -> chain 3 (w_hi)
        u.require_inp0 = ENABLE
        u.require_inp1 = ENABLE
        dp = u.datapath_config
        if kind in ("u1", "u2"):
            dp[0].enable_alu(AluOp.MULTIPLY, AluInp(PD + 0), AluInp(PD + 1))
            dp[0].pass_through_delay(2, 3)  # x_hi, w_hi onward to st1's mul
            dp[1].enable_alu(AluOp.MULTIPLY, AluInp(PD + 2), AluInp(PD + 3))
            dp[1].enable_delay_from_src(DelayInp.PREV_ALU_OUT, 4)
            dp[2].enable_alu(AluOp.ADD, AluInp.PREV_ALU_OUT, AluInp(PD + 4))
            if kind == "u2":
                # carry x_lo to st3, x_hi to st4 for the latches
                dp[0].pass_through_delay(0)
                dp[1].pass_through_delay(0, 2)
                dp[2].pass_through_delay(0, 2)
                dp[3].enable_alu(
                    AluOp.BYPASS, AluInp.PREV_ALU_OUT, AluInp(PD + 0)
                )
                dp[3].swap_enable = ENABLE  # swap@st3 <- x2
                dp[3].pass_through_delay(2)
                dp[4].enable_alu(
                    AluOp.BYPASS, AluInp.PREV_ALU_OUT, AluInp(PD + 2)
                )
                dp[4].swap_enable = ENABLE  # swap@st4 <- x3
            else:
                dp[3].pass_through_alu()
                dp[4].pass_through_alu()
            dp[5].pass_through_alu()
            if kind == "u1":
                dp[6].enable_alu(
                    AluOp.BYPASS, AluInp.PREV_ALU_OUT, AluInp.PREV_ALU_OUT
                )  # A <- s01
            else:
                dp[6].enable_alu(
                    AluOp.ADD, AluInp.CURR_ALU_OUT, AluInp.PREV_ALU_OUT
                )  # A += s23
        elif kind == "u3":
            # st0: A4 = x4*wA4; st1: B2 = x4*wB2 (x_lo reused on both muls)
            dp[0].enable_alu(AluOp.MULTIPLY, AluInp(PD + 0), AluInp(PD + 1))
            dp[0].pass_through_delay(0, 2, 3)
            dp[1].enable_alu(AluOp.MULTIPLY, AluInp(PD + 0), AluInp(PD + 3))
            dp[1].enable_delay_from_src(DelayInp.PREV_ALU_OUT, 4)  # A4
            dp[1].pass_through_delay(2)
            for st in (2, 3, 4):
                dp[st].pass_through_alu()  # pass B2 down
                dp[st].pass_through_delay(2, 4)
            dp[5].enable_alu(
                AluOp.BYPASS, AluInp.PREV_ALU_OUT, AluInp(PD + 2)
            )
            dp[5].swap_enable = ENABLE  # swap@st5 <- x5
            dp[5].pass_through_delay(4)
            dp[6].enable_alu(AluOp.ADD, AluInp.CURR_ALU_OUT, AluInp(PD + 4))
            dp[6].enable_delay_from_src(DelayInp.PREV_ALU_OUT, 5)  # B2
            dp[7].enable_alu(AluOp.BYPASS, AluInp(PD + 5), AluInp(PD + 5))
            # st7 out flop <- B2 (B seed)
        elif kind == "u4":
            # w pair (wB0, wB1); x pair (x6, x7): latch x6, x7 unused.
            dp[0].pass_through_delay(0, 1, 3)
            dp[1].pass_through_delay(0, 1, 3)
            dp[2].enable_alu(
                AluOp.BYPASS, AluInp.PREV_ALU_OUT, AluInp(PD + 0)
            )
            dp[2].swap_enable = ENABLE  # swap@st2 <- x6
            dp[2].pass_through_delay(1, 3)
            dp[3].enable_alu(AluOp.MULTIPLY, AluInp.CURR_SWAP_OUT, AluInp(PD + 1))
            dp[3].pass_through_delay(3)  # B0 = x2*wB0
            dp[4].enable_alu(AluOp.MULTIPLY, AluInp.CURR_SWAP_OUT, AluInp(PD + 3))
            dp[4].enable_delay_from_src(DelayInp.PREV_ALU_OUT, 4)  # B0
            # B1 = x3*wB1
            dp[5].enable_alu(AluOp.ADD, AluInp.PREV_ALU_OUT, AluInp(PD + 4))
            dp[6].enable_delay_from_src(DelayInp.PREV_ALU_OUT, 5)  # s; A held
            dp[7].enable_alu(AluOp.ADD, AluInp.CURR_ALU_OUT, AluInp(PD + 5))
            dp[7].enable_delay_from_src(DelayInp.PREV_ALU_OUT, 5)  # A -> lane5@st7
        elif kind == "u5":
            # w pair (wB3, wB4); x pair consumed but unused.
            dp[0].pass_through_delay(1, 3)
            dp[1].pass_through_delay(1, 3)
            dp[2].enable_alu(AluOp.MULTIPLY, AluInp.CURR_SWAP_OUT, AluInp(PD + 3))
            dp[2].pass_through_delay(1)  # B4 = x6*wB4
            dp[3].pass_through_alu()
            dp[3].pass_through_delay(1)
            dp[4].pass_through_alu()
            dp[4].pass_through_delay(1)
            dp[5].enable_alu(AluOp.MULTIPLY, AluInp.CURR_SWAP_OUT, AluInp(PD + 1))
            dp[5].enable_delay_from_src(DelayInp.PREV_ALU_OUT, 4)  # B4
            # B3 = x5*wB3
            dp[6].enable_alu(AluOp.ADD, AluInp.PREV_ALU_OUT, AluInp(PD + 4))
            # s2 = B3+B4 (clobbers A flop; A already stashed in lane5@st7)
            dp[7].enable_alu(AluOp.ADD, AluInp.CURR_ALU_OUT, AluInp.PREV_ALU_OUT)
            u.enable_output(OutSel.DELAY_5, OutPath.WR0_LO)  # A
            u.enable_output(OutSel.ALU_OUT, OutPath.WR0_HI)  # B
        return u

    def _chain2x(u, succ):
        u.trigger = (Trigger.SRC_TENSOR_DONE, Trigger.COUNT, Trigger.NONE)
        u.next_uop = (0, succ, 0)
        u.repeat_count = 1
        return u

    # index 0 is the entry copy of u1 (index 0 is also IDLE as a next_uop
    # target, so the loop body lives at 1..5)
    two_uops = [
        _chain2x(_u2x("u1"), 2),
        _chain2x(_u2x("u1"), 2),
        _chain2x(_u2x("u2"), 3),
        _chain2x(_u2x("u3"), 4),
        _chain2x(_u2x("u4"), 5),
        _chain2x(_u2x("u5"), 1),
    ]

    # --- 1X program (fallback; also what runs if alignment breaks) ---------
    # Chains: 0 = x, 1 = w, 4/5 = captures. Swap latches: x2@st1, x3@st2,
    # x4@st3, x5@st4, x6@st5. A accumulates in st6, B in st7. A is emitted
    # at i4 (via st7 bypass), B at i9.
    def _u1x(kind):
        u = UopConfig()
        u.enable_input(InpSel.SRC_0, 1)  # -> chain 0 (x)
        u.enable_input(InpSel.SRC_1, 2)  # -> chain 1 (w)
        u.require_inp0 = ENABLE
        u.require_inp1 = ENABLE
        dp = u.datapath_config
        if kind in ("i0", "i1", "i2", "i3", "i4"):
            dp[0].enable_alu(AluOp.MULTIPLY, AluInp(PD + 0), AluInp(PD + 1))
            latch_st = {"i2": 1, "i3": 2, "i4": 3}.get(kind)
            if latch_st is not None:
                for st in range(latch_st):
                    dp[st].pass_through_delay(0)
            for st in range(1, 6):
                if st == latch_st:
                    dp[st].enable_alu(
                        AluOp.BYPASS, AluInp.PREV_ALU_OUT, AluInp(PD + 0)
                    )
                    dp[st].swap_enable = ENABLE
                else:
                    dp[st].pass_through_alu()
            if kind == "i0":
                dp[6].enable_alu(
                    AluOp.BYPASS, AluInp.PREV_ALU_OUT, AluInp.PREV_ALU_OUT
                )
            else:
                dp[6].enable_alu(
                    AluOp.ADD, AluInp.CURR_ALU_OUT, AluInp.PREV_ALU_OUT
                )
            if kind == "i4":
                # A final: mirror it into st7's flop and emit
                dp[7].enable_alu(
                    AluOp.BYPASS, AluInp.PREV_ALU_OUT, AluInp.PREV_ALU_OUT
                )
                u.enable_output(OutSel.ALU_OUT, OutPath.WR0_LO)
        elif kind == "i5":
            # B2 = x4*wB2 at st3; latch x5@st4; B <- B2 (seed)
            for st in (0, 1, 2):
                dp[st].pass_through_delay(0, 1)
            dp[3].enable_alu(AluOp.MULTIPLY, AluInp.CURR_SWAP_OUT, AluInp(PD + 1))
            dp[3].pass_through_delay(0)
            dp[4].enable_alu(
                AluOp.BYPASS, AluInp.PREV_ALU_OUT, AluInp(PD + 0)
            )
            dp[4].swap_enable = ENABLE
            dp[5].pass_through_alu()
            dp[6].enable_delay_from_src(DelayInp.PREV_ALU_OUT, 5)  # B2; A held
            dp[7].enable_alu(AluOp.BYPASS, AluInp(PD + 5), AluInp(PD + 5))
        elif kind == "i6":
            # B0 = x2*wB0 at st1; latch x6@st5; B += B0
            dp[0].pass_through_delay(0, 1)
            dp[1].enable_alu(AluOp.MULTIPLY, AluInp.CURR_SWAP_OUT, AluInp(PD + 1))
            dp[1].pass_through_delay(0)
            for st in (2, 3, 4):
                dp[st].pass_through_alu()
                dp[st].pass_through_delay(0)
            dp[5].enable_alu(
                AluOp.BYPASS, AluInp.PREV_ALU_OUT, AluInp(PD + 0)
            )
            dp[5].swap_enable = ENABLE
            dp[6].enable_delay_from_src(DelayInp.PREV_ALU_OUT, 5)
            dp[7].enable_alu(AluOp.ADD, AluInp.CURR_ALU_OUT, AluInp(PD + 5))
        elif kind in ("i7", "i8", "i9"):
            mul_st = {"i7": 2, "i8": 4, "i9": 5}[kind]
            for st in range(mul_st):
                dp[st].pass_through_delay(1)
            dp[mul_st].enable_alu(
                AluOp.MULTIPLY, AluInp.CURR_SWAP_OUT, AluInp(PD + 1)
            )
            for st in range(mul_st + 1, 6):
                dp[st].pass_through_alu()
            dp[6].enable_delay_from_src(DelayInp.PREV_ALU_OUT, 5)
            dp[7].enable_alu(AluOp.ADD, AluInp.CURR_ALU_OUT, AluInp(PD + 5))
            if kind == "i9":
                u.enable_output(OutSel.ALU_OUT, OutPath.WR0_LO)
        return u

    def _chain1x(u, succ):
        u.trigger = (Trigger.SRC_TENSOR_DONE, Trigger.COUNT, Trigger.NONE)
        u.next_uop = (0, succ, 0)
        u.repeat_count = 1
        return u

    kinds1x = ["i0", "i0", "i1", "i2", "i3", "i4", "i5", "i6", "i7", "i8", "i9"]
    base_uops = [
        _chain1x(_u1x(k), 2 if idx == 0 else (idx + 1) if idx < 10 else 1)
        for idx, k in enumerate(kinds1x)
    ]
    # table-gen requires equal variant lengths; pad 2x with unreachable slots
    two_uops = two_uops + [UopConfig() for _ in range(len(base_uops) - len(two_uops))]

    def _ref(in0, in1, s0, s1, imm2):
        p = in0.shape[0]
        x = np.asarray(in0, np.float32).reshape(p, -1, 10)
        w = np.asarray(in1, np.float32).reshape(p, -1, 10)
        a = (x[..., 0:5] * w[..., 0:5]).sum(axis=-1)
        b = (
            x[..., 4] * w[..., 5]
            + x[..., 2] * w[..., 6]
            + x[..., 3] * w[..., 7]
            + x[..., 5] * w[..., 8]
            + x[..., 6] * w[..., 9]
        )
        out = np.stack([a, b], axis=-1).reshape(p, -1)
        return out

    spec = Spec(body=scan(SAluOp.ADD, Src0 * Src1), reference=_ref)
    if name in dve_ops._SUB_OPCODE_FOR_NAME:
        row = dve_ops._SUB_OPCODE_FOR_NAME[name]
        op = next(o for o in dve_ops.OPS if o.name == name)
    else:
        row = dve_ops._CUSTOM_DVE_ROW_BASE + len(dve_ops.OPS)
        assert row < 0x20
        op = None
    shas = {}
    for ver in {dve_ver_for("TRN2"), "v3", "v4"}:
        compiled = DveOpSpec(
            name=name,
            opcode=row,
            uops=base_uops,
            uops_2x=two_uops,
            rd1_en=True,
            perf_max=1,
        )
        dve_ops._COMPILE_CACHE[(name, ver)] = compiled
        shas[ver] = compiled.sha(ver)
    if op is None:
        op = dve_ops.DveOp(name, spec, subdim=True, uops_sha=shas)
        dve_ops.OPS.append(op)
        dve_ops.CUSTOM_DVE_SPECS[name] = spec
        dve_ops._SUB_OPCODE_FOR_NAME[name] = row
    else:
        op.uops_sha.clear()
        op.uops_sha.update(shas)
    return op


def _set_perf_max(nc, val):
    for fn in nc.m.functions:
        for bb in fn.blocks:
            for inst in bb.instructions:
                if isinstance(inst, mybir.InstCustomDveAnt):
                    inst.perf_max = val


N, C, H, W = 4, 64, 128, 128
K, PAD, CW = 5, 2, 8
HO, WO = 128, 128
RH = 64
WP = W + 2 * PAD  # 132
NJ, NPC, RB = 8, 16, 4
QP = RB * WP // 2  # 264 pixels per parity per partition per group
NPAIR = QP // 2  # 132 pixel pairs
WSEG = 10  # weights per pair (pad-free)
WFREE = NPAIR * WSEG  # 1320
XL = (RB + K) * WP  # 1188
GG = 2  # channel groups merged per scan (weights re-streamed via stride-0)
NGG = 8 // GG  # merged blocks per parity
XW = 4 * NPAIR  # per-group x window inside a merged tile (528)
XQ = XW + XW + 8  # merged two-group x window (1064)
XV = 4 * XW + 8  # merged four-group x window (2120)
F16 = mybir.dt.float16

PHASE = int(os.environ.get("LC_PHASE", "2"))


def _build_program(phase=PHASE, repeat=1):
    nc = bass.Bass()
    mac = _register_mac_pair10()
    # xq: per (par, i, gg) a two-group merged x window; group 2*gg's 528
    # elements then group 2*gg+1's window (534 + pad). Only the slots for
    # pair-sized blocks are loaded (par0 gg0/gg1, par1 gg2/gg3).
    xq_d = nc.declare_dram_parameter(
        "xq", [2, K, NGG, 128, XQ], F16, isOutput=False
    )
    # xv: four-group merged windows for the mid-run quad blocks;
    # slot 0 = (par0, groups 4-7), slot 1 = (par1, groups 0-3)
    xv_d = nc.declare_dram_parameter("xv", [2, K, 128, XV], F16, isOutput=False)
    ws_d = nc.declare_dram_parameter("ws", [K, 2, 128, WFREE], F16, isOutput=False)
    out_d = nc.declare_dram_parameter("out", [8, 2, 128, QP], F16, isOutput=True)

    xq_a = xq_d[:]
    xv_a = xv_d[:]
    ws_a = ws_d[:]
    out_a = out_d[:]

    with tile.TileContext(nc) as tc:
        with (
            tc.tile_pool(name="wpool", bufs=1) as wpool,
            tc.tile_pool(name="xpool", bufs=1) as xpool,
            tc.tile_pool(name="opool", bufs=4) as opool,
            tc.tile_pool(name="oqpool", bufs=2) as oqpool,
            tc.tile_pool(name="ogpool", bufs=4) as ogpool,
            tc.tile_pool(name="tpool", bufs=3) as tpool,
        ):
            # All input DMAs are pre-emitted in first-use order (blocks run
            # parity-major); w(i) and xq(i, gg0/gg1) interleave so the first
            # two merged blocks' scans are paced by arrivals
            w_tiles = {}
            x_tiles = {}

            def _load_w(i, par):
                wt = wpool.tile([128, WFREE], F16, tag=f"w{i}_{par}")
                nc.sync.dma_start(
                    wt[:],
                    ws_a.__replace__(
                        ap=[[WFREE, 128], [1, WFREE]],
                        offset=(i * 2 + par) * 128 * WFREE,
                    ),
                )
                w_tiles[(i, par)] = wt

            def _load_xq(par, i, gg):
                xt = xpool.tile([128, XQ], F16, tag=f"xq{par}_{i}_{gg}")
                nc.sync.dma_start(
                    xt[:],
                    xq_a.__replace__(
                        ap=[[XQ, 128], [1, XQ]],
                        offset=((par * K + i) * NGG + gg) * 128 * XQ,
                    ),
                )
                x_tiles[(par, i, gg)] = xt

            xv_tiles = {}

            def _load_xv(slot, i):
                xt = xpool.tile([128, XV], F16, tag=f"xv{slot}_{i}")
                nc.sync.dma_start(
                    xt[:],
                    xv_a.__replace__(
                        ap=[[XV, 128], [1, XV]],
                        offset=(slot * K + i) * 128 * XV,
                    ),
                )
                xv_tiles[(slot, i)] = xt

            _load_w(0, 0)
            _load_xq(0, 0, 0)
            _load_w(1, 0)
            _load_xq(0, 1, 0)
            _load_xq(0, 0, 1)
            for i in range(2, K):
                _load_w(i, 0)
                _load_xq(0, i, 0)
                _load_xq(0, i - 1, 1)
            _load_xq(0, K - 1, 1)
            for i in range(K):
                _load_xv(0, i)
            for i in range(K):
                _load_w(i, 1)
            for i in range(K):
                _load_xv(1, i)
            for gg in (2, 3):
                for i in range(K):
                    _load_xq(1, i, gg)

            def _scan(par, gg, i, gsub=None):
                """Merged two-group scan, or a single group of the merged
                tile when gsub is given (used to keep the tail combine
                small on the final block)."""
                if gsub is None:
                    o = opool.tile([128, 2 * QP], F16, tag=f"o{par}_{i}")
                    npair, xoff, woff, ov = GG * NPAIR, 0, 0, None
                else:
                    o = opool.tile([128, QP], F16, tag=f"os{par}_{i}")
                    npair, xoff = NPAIR, gsub * XW
                xa = x_tiles[(par, i, gg)][:]
                in0 = xa.__replace__(
                    ap=[xa.ap[0], [4, npair], [1, WSEG]],
                    offset=xa.offset + xoff,
                )
                wa = w_tiles[(i, par)][:]
                wap = (
                    [wa.ap[0], [1, WFREE]]
                    if gsub is not None
                    else [wa.ap[0], [0, GG], [1, WFREE]]
                )
                in1 = wa.__replace__(ap=wap, offset=wa.offset)
                nc.vector._custom_dve(mac, out=o[:], in0=in0, in1=in1)
                return o

            def _scan_quad(slot, par, i):
                o = oqpool.tile([128, 4 * QP], F16, tag=f"oq{i}")
                xa = xv_tiles[(slot, i)][:]
                in0 = xa.__replace__(
                    ap=[xa.ap[0], [4, 4 * NPAIR], [1, WSEG]],
                    offset=xa.offset,
                )
                wa = w_tiles[(i, par)][:]
                in1 = wa.__replace__(
                    ap=[wa.ap[0], [0, 4], [1, WFREE]], offset=wa.offset
                )
                nc.vector._custom_dve(mac, out=o[:], in0=in0, in1=in1)
                return o

            # first two even-parity merged blocks: interleave scan emission
            # so block-1 scans fill block-0's weight-arrival stalls
            pre = {0: [], 1: []}
            order = [(0, 0), (0, 1), (1, 0), (0, 2), (1, 1), (0, 3), (1, 2),
                     (0, 4), (1, 3), (1, 4)]
            for gg, i in order:
                pre[gg].append((i, _scan(0, gg, i)))
            for gg in (0, 1):
                pre[gg] = [o for _, o in sorted(pre[gg])]

            def _store(par, g, og, col):
                nc.sync.dma_start(
                    out_a.__replace__(
                        ap=[[QP, 128], [1, QP]],
                        offset=(g * 2 + par) * 128 * QP,
                    ),
                    og[:, col : col + QP],
                )

            def _quad_block(slot, par, gbase):
                os_ = [_scan_quad(slot, par, i) for i in range(K)]
                # combine in two pair-wide chunks so Pool granularity and
                # the store pattern match the pair blocks
                for c in range(2):
                    lo, hi = c * 2 * QP, (c + 1) * 2 * QP
                    og = ogpool.tile([128, 2 * QP], F16, tag=f"og{par}")
                    t2 = tpool.tile([128, 2 * QP], F16, tag=f"t{par}")
                    nc.gpsimd.tensor_add(
                        og[:, :], os_[0][:, lo:hi], os_[1][:, lo:hi]
                    )
                    nc.gpsimd.tensor_add(
                        t2[:, :], os_[2][:, lo:hi], os_[3][:, lo:hi]
                    )
                    nc.gpsimd.tensor_add(og[:, :], og[:, :], t2[:, :])
                    nc.gpsimd.tensor_add(og[:, :], og[:, :], os_[4][:, lo:hi])
                    for s in range(2):
                        _store(par, gbase + 2 * c + s, og, s * QP)

            for par in range(2):
                if par == 1:
                    # odd parity: quad block (groups 0-3) first, then the
                    # pair block (4,5), then the single-group tail (6,7)
                    _quad_block(1, 1, 0)
                for gg in range(repeat * NGG):
                    gg = gg % NGG
                    if par == 0 and gg in (2, 3):
                        if gg == 2:
                            _quad_block(0, 0, 4)
                        continue
                    if par == 1 and gg in (0, 1):
                        continue
                    if par == 1 and gg == NGG - 1:
                        # final block: two single-group sub-blocks so only a
                        # narrow combine trails the last scan; the first
                        # sub-block's combine runs on Pool in parallel
                        for gsub in range(GG):
                            og = ogpool.tile([128, QP], F16, tag="ogf")
                            if gsub == GG - 1:
                                # combine incrementally on the DVE between
                                # scans; one add + store trail the last scan
                                o0 = _scan(par, gg, 0, gsub)
                                o1 = _scan(par, gg, 1, gsub)
                                nc.vector.tensor_add(
                                    og[:, :], o0[:, :], o1[:, :]
                                )
                                for i in range(2, K):
                                    oi = _scan(par, gg, i, gsub)
                                    nc.vector.tensor_add(
                                        og[:, :], og[:, :], oi[:, :]
                                    )
                            else:
                                os_ = [
                                    _scan(par, gg, i, gsub) for i in range(K)
                                ]
                                t2 = tpool.tile([128, QP], F16, tag="tf")
                                nc.gpsimd.tensor_add(
                                    og[:, :], os_[0][:, :], os_[1][:, :]
                                )
                                nc.gpsimd.tensor_add(
                                    t2[:, :], os_[2][:, :], os_[3][:, :]
                                )
                                nc.gpsimd.tensor_add(
                                    og[:, :], og[:, :], t2[:, :]
                                )
                                nc.gpsimd.tensor_add(
                                    og[:, :], og[:, :], os_[4][:, :]
                                )
                            _store(par, gg * GG + gsub, og, 0)
                        continue
                    og = ogpool.tile([128, 2 * QP], F16, tag=f"og{par}")
                    if par == 0 and gg in (0, 1):
                        os_ = pre[gg]
                    else:
                        os_ = [_scan(par, gg, i) for i in range(K)]
                    t2 = tpool.tile([128, 2 * QP], F16, tag=f"t{par}")
                    eng = nc.gpsimd
                    eng.tensor_add(og[:, :], os_[0][:, :], os_[1][:, :])
                    eng.tensor_add(t2[:, :], os_[2][:, :], os_[3][:, :])
                    eng.tensor_add(og[:, :], og[:, :], t2[:, :])
                    eng.tensor_add(og[:, :], og[:, :], os_[4][:, :])
                    for gsub in range(GG):
                        _store(par, gg * GG + gsub, og, gsub * QP)
    _set_perf_max(nc, 1)
    mybir.codegen_inst_isa_subclasses(nc)
    _split_multi_waits(nc)
    return nc


def _shard_inputs(input, weight):
    input = np.asarray(input, dtype=np.float32)
    weight = np.asarray(weight, dtype=np.float32)
    in_maps = []
    for n in range(N):
        xp = np.pad(input[n], ((0, 0), (PAD, PAD + 1), (PAD, PAD)))  # [64,133,132]
        sw = np.lib.stride_tricks.sliding_window_view(xp, (RB + K), axis=1)
        sw = np.transpose(sw, (0, 1, 3, 2))  # [c, row0, 9, 132]
        wv = weight[n].reshape(NJ, K, K, HO, WO)
        for half in range(2):
            r0 = RH * half
            idx = r0 + np.arange(NPC) * RB
            slab = sw[:, idx]  # [64, 16, 9, 132]
            xe = np.ascontiguousarray(slab.reshape(C // CW, NJ, NPC, XL))
            xe = xe.reshape(8, 128, XL)
            xo = np.zeros_like(xe)
            xo[..., :-1] = xe[..., 1:]
            xs = np.stack([xe, xo], axis=1).astype(np.float16)  # [8, 2, 128, XL]

            warr = wv[:, :, :, r0 : r0 + RH, :].reshape(
                NJ, K, K, NPC, RB, WO // 2, 2
            )  # [j, i, jj, pc, rr, m, par]
            # taps per (i, par, j, pc, rr, within-parity pixel m, col-tap jj)
            wt = np.zeros((K, 2, NJ, NPC, RB, WP // 2, K), np.float32)
            wt[:, :, :, :, :, : WO // 2, :] = np.transpose(
                warr, (1, 6, 0, 3, 4, 5, 2)
            )
            # pack per pixel-pair: [A0..A4, B2, B0, B1, B3, B4]
            wpair = wt.reshape(K, 2, NJ, NPC, RB, WP // 4, 2, K)
            wpp = np.empty((K, 2, NJ, NPC, RB, WP // 4, WSEG), np.float32)
            wpp[..., 0:5] = wpair[..., 0, :]
            wpp[..., 5] = wpair[..., 1, 2]
            wpp[..., 6] = wpair[..., 1, 0]
            wpp[..., 7] = wpair[..., 1, 1]
            wpp[..., 8] = wpair[..., 1, 3]
            wpp[..., 9] = wpair[..., 1, 4]
            ws = wpp.reshape(K, 2, 128, WFREE).astype(np.float16)

            # merged two-group x windows: xq[par, i, gg] =
            #   [group 2gg's window (528) | group 2gg+1's window (536)]
            xq = np.zeros((2, K, NGG, 128, XQ), np.float16)
            for par in range(2):
                for i in range(K):
                    for gg in range(NGG):
                        xq[par, i, gg, :, :XW] = xs[
                            2 * gg, par, :, WP * i : WP * i + XW
                        ]
                        xq[par, i, gg, :, XW : XW + XW + 8] = xs[
                            2 * gg + 1, par, :, WP * i : WP * i + XW + 8
                        ]
            # merged four-group windows for the quad blocks:
            # slot 0 = (par 0, groups 4-7), slot 1 = (par 1, groups 0-3)
            xv = np.zeros((2, K, 128, XV), np.float16)
            for slot, (par, gb) in enumerate(((0, 4), (1, 0))):
                for i in range(K):
                    for s in range(4):
                        wlen = XW + 8 if s == 3 else XW
                        xv[slot, i, :, s * XW : s * XW + wlen] = xs[
                            gb + s, par, :, WP * i : WP * i + wlen
                        ]
            in_maps.append({"xq": xq, "xv": xv, "ws": ws})
    return in_maps


def kernel(input, weight):
    nc = _build_program(PHASE)
    in_maps = _shard_inputs(input, weight)
    res = run_bass_kernel_spmd(nc, in_maps, list(range(8)))
    out = np.empty((N, C, HO, WO), dtype=np.float32)
    for k in range(8):
        n, half = divmod(k, 2)
        o = np.asarray(res.results[k]["out"], dtype=np.float32)
        o = o.reshape(8, 2, NJ, NPC, RB, WP // 2)[..., : WO // 2]
        o = np.transpose(o, (0, 2, 3, 4, 5, 1))  # [g, j, pc, rr, m, par]
        out[n, :, RH * half : RH * (half + 1), :] = o.reshape(C, RH, WO)
    return out
